# revision 19
# baseline (speedup 1.0000x reference)
"""Trainium2 Bass kernel for nn_CalAttenMap (gnn message passing + dense softmax).

Strategy (8 NeuronCores, no collectives):
  - Edges partitioned on host by destination row block (a = pair_idxs[:,0]);
    core c owns output rows [256c, 256(c+1)).  Duplicate (a,b) pairs merged
    host-side (union rows summed, bias multiplicity tracked) so the device
    scatter is a pure write.
  - Softmax identity: the dense logit tensor is 0 except edge cells (and
    -1e4 diag), so softmax(axis=1) = 1/Z on background cells, exp(af)/Z on
    edges, 0 on diag, with Z(i,p) = (N-1-cnt_i) + sum_edges exp(af).  The
    256MB dense tensor is never materialized: each core broadcast-fills its
    32MB block with 1/Z and scatters edge values over it.
  - Per-core device pipeline (macroblocks of 512 edges, d-major):
      s,o projections on PE (fp32); o split to bf16 hi/lo in DRAM
      o rows gathered PRE-TRANSPOSED via dma_gather(transpose=True) on the
        hi/lo parts and recombined to fp32 (no PE transposes, full accuracy)
      s rows gathered via one-hot matmul (a is sorted; s lives in SBUF)
      union streamed pre-transposed (host layout)
      prodT = sT*oT*uT on DVE; af.T = wwT.T @ prodT on PE (N=512 moving)
      bias+gate folded into one K=2 matmul (mult*w_b + loggate rows)
      exp on ACT; transpose back [16,128]->[128,16] on PE; Z row-sums via
        one-hot matmul into PSUM; 1/Z on DVE
      fill (broadcast DMA) + per-128-edge indirect scatter of exp(af)/Z
"""

import numpy as np

import concourse.bass as bass
import concourse.bacc as bacc
import concourse.mybir as mybir
import concourse.tile as tile
from concourse import bass_utils
from concourse.bass_interp import get_hw_module

F32 = mybir.dt.float32
BF16 = mybir.dt.bfloat16
I32 = mybir.dt.int32
I16 = mybir.dt.int16

N = 2048          # nodes
D = 512           # feature dim
P = 16            # heads per pair
N_CORES = 8
R = N // N_CORES  # rows per core (256)
G = R // 128      # row groups per core (2)
KC = D // 128     # contraction chunks (4)
MBE = 512         # edges per macroblock

LAST_RESULTS = None


# --------------------------------------------------------------------------
# host prep
# --------------------------------------------------------------------------

def _host_prep(obj_feats, union_feats, pair_idxs):
    a = pair_idxs[:, 0].astype(np.int64)
    b = pair_idxs[:, 1].astype(np.int64)
    key = a * N + b
    order = np.argsort(key, kind="stable")
    ks = key[order]
    uniq_mask = np.ones(len(ks), bool)
    if len(ks) > 1:
        uniq_mask[1:] = ks[1:] != ks[:-1]
    starts = np.nonzero(uniq_mask)[0]
    ku = ks[starts]
    mult = np.diff(np.append(starts, len(ks))).astype(np.float32)
    u_merged = np.add.reduceat(union_feats[order], starts, axis=0).astype(np.float32)

    au = ku // N
    bu = ku % N
    gate = (au != bu)

    cnt = np.bincount(au[gate], minlength=N)
    base = (N - 1 - cnt).astype(np.float32)

    diag = np.arange(N, dtype=np.int64)
    A = np.concatenate([au, diag])
    B_ = np.concatenate([bu, diag])
    LG = np.concatenate([np.where(gate, 0.0, -30000.0).astype(np.float32),
                         np.full(N, -30000.0, np.float32)])
    MU = np.concatenate([mult, np.zeros(N, np.float32)])
    UI = np.concatenate([np.arange(len(ku), dtype=np.int64), np.full(N, -1)])
    o2 = np.argsort(A, kind="stable")
    A, B_, LG, MU, UI = A[o2], B_[o2], LG[o2], MU[o2], UI[o2]

    n_groups = N // 128
    counts = np.bincount(A // 128, minlength=n_groups)
    # blocks per group: uniform across cores, multiple of 4 (macroblocks)
    BB = max(4, int(-(-counts.max() // 128)))
    BB = (BB + 3) // 4 * 4
    cap = BB * 128

    g_start = np.zeros(n_groups + 1, np.int64)
    g_start[1:] = np.cumsum(counts)

    e_tot = n_groups * cap
    Ap = np.empty(e_tot, np.int64)
    Bp = np.zeros(e_tot, np.int64)
    LGp = np.full(e_tot, -30000.0, np.float32)
    MUp = np.zeros(e_tot, np.float32)
    UIp = np.full(e_tot, -1, np.int64)
    for t in range(n_groups):
        s0, s1 = g_start[t], g_start[t + 1]
        d0 = t * cap
        n_t = s1 - s0
        Ap[d0:d0 + n_t] = A[s0:s1]
        Bp[d0:d0 + n_t] = B_[s0:s1]
        LGp[d0:d0 + n_t] = LG[s0:s1]
        MUp[d0:d0 + n_t] = MU[s0:s1]
        UIp[d0:d0 + n_t] = UI[s0:s1]
        Ap[d0 + n_t:d0 + cap] = t * 128
        Bp[d0 + n_t:d0 + cap] = t * 128  # pad writes 0 to group's 1st diag cell

    MB = G * BB // 4  # macroblocks per core
    per_core = []
    for c in range(N_CORES):
        sl = slice(c * G * cap, (c + 1) * G * cap)
        Ac, Bc, LGc, MUc, UIc = Ap[sl], Bp[sl], LGp[sl], MUp[sl], UIp[sl]
        a_loc = Ac - c * R
        E = G * cap
        union_c = np.zeros((E, D), np.float32)
        has_u = UIc >= 0
        union_c[has_u] = u_merged[UIc[has_u]]
        # pre-transposed union: [MB, KC, 128, 512]
        uT = np.ascontiguousarray(
            union_c.reshape(MB, MBE, KC, 128).transpose(0, 2, 3, 1))
        # dma_gather index packing: idx i -> row i%16, col i//16, tiled to 128
        bp = np.ascontiguousarray(
            np.tile(Bc.reshape(MB, MBE // 16, 16).transpose(0, 2, 1), (1, 8, 1))
            .transpose(1, 0, 2).reshape(128, MB * (MBE // 16)).astype(np.int16))
        nb = G * BB
        per_core.append(dict(
            uT=uT,
            b_pack=bp,
            a_row=np.ascontiguousarray((Ac % 128).astype(np.float32).reshape(1, E)),
            a_relT=np.ascontiguousarray(
                (Ac % 128).astype(np.float32).reshape(nb, 128).T),
            bg=np.ascontiguousarray(np.stack([MUc, LGc]).astype(np.float32)),
            cellT=np.ascontiguousarray(
                ((Ac % 128) * N + Bc).reshape(nb, 128).T.astype(np.int32)),
            base=np.ascontiguousarray(base[c * R:(c + 1) * R].reshape(G, 128).T),
            objT_own=np.ascontiguousarray(obj_feats[c * R:(c + 1) * R].T),
        ))
    return BB, per_core


# --------------------------------------------------------------------------
# device program
# --------------------------------------------------------------------------

def _build_program(BB):
    nc = bacc.Bacc("TRN2", target_bir_lowering=False, debug=False,
                   enable_asserts=True, num_devices=N_CORES)

    nb = G * BB
    MB = nb // 4          # macroblocks per core
    MBG = MB // G         # macroblocks per group
    E = nb * 128

    dt_in = lambda name, shape, dt=F32: nc.dram_tensor(
        name, shape, dt, kind="ExternalInput").ap()

    objT_own = dt_in("objT_own", [D, R])
    wsT = dt_in("wsT", [D, D])
    woT = dt_in("woT", [D, D])
    wwT = dt_in("wwT", [D, P])
    ws_b = dt_in("ws_b", [1, D])
    wo_b = dt_in("wo_b", [1, D])
    bg_lhs = dt_in("bg_lhs", [2, P])
    ones = dt_in("ones", [1, 128])
    colidx = dt_in("colidx", [128, 128])
    rowidx = dt_in("rowidx", [128, 1])
    ident16 = dt_in("ident16", [16, 16])
    base_d = dt_in("base", [128, G])
    uT_d = dt_in("uT", [MB, KC, 128, MBE])
    b_pack = dt_in("b_pack", [128, MB * (MBE // 16)], I16)
    a_row = dt_in("a_row", [1, E])
    a_relT = dt_in("a_relT", [128, nb])
    bg_d = dt_in("bg", [2, E])
    cellT = dt_in("cellT", [128, nb], I32)

    cc_in = nc.dram_tensor("cc_in", [R, 2 * D], BF16, kind="Internal").ap()
    hilo_dram = nc.dram_tensor("hilo_scr", [N, 2 * D], BF16, kind="Internal",
                               addr_space="Shared").ap()
    out_ds = [nc.dram_tensor(f"out{g}", [128 * N, P], F32, kind="ExternalOutput").ap()
              for g in range(G)]

    eq = mybir.AluOpType.is_equal
    SUB = mybir.AluOpType.subtract
    EXP = mybir.ActivationFunctionType.Exp

    with tile.TileContext(nc) as tc:
        with tc.tile_pool(name="const", bufs=1) as cp:
            wwT_sb = cp.tile([128, KC * P], F32)
            nc.sync.dma_start(wwT_sb[:].rearrange("p (c n) -> p c n", c=KC),
                              wwT.rearrange("(c p) n -> p c n", p=128))
            bgl_sb = cp.tile([2, P], F32)
            nc.sync.dma_start(bgl_sb[:], bg_lhs[:])
            colidx_sb = cp.tile([128, 128], F32)
            nc.sync.dma_start(colidx_sb[:], colidx[:])
            rowidx_sb = cp.tile([128, 1], F32)
            nc.sync.dma_start(rowidx_sb[:], rowidx[:])
            id16_sb = cp.tile([16, 16], F32)
            nc.sync.dma_start(id16_sb[:], ident16[:])
            base_sb = cp.tile([128, G], F32)
            nc.sync.dma_start(base_sb[:], base_d[:])
            bpack_sb = cp.tile([128, MB * (MBE // 16)], I16)
            nc.sync.dma_start(bpack_sb[:], b_pack[:])
            a_relT_sb = cp.tile([128, nb], F32)
            nc.sync.dma_start(a_relT_sb[:], a_relT[:])
            cellT_sb = cp.tile([128, nb], I32)
            nc.sync.dma_start(cellT_sb[:], cellT[:])
            s_hi = cp.tile([128, G * D], BF16)
            s_lo = cp.tile([128, G * D], BF16)

            # ---------------- stage A: projections ----------------
            with tc.tile_pool(name="proj_psum", bufs=2, space="PSUM") as pp, \
                 tc.tile_pool(name="aconst", bufs=1) as ac, \
                 tc.tile_pool(name="proj_sb", bufs=3) as ps:
                objTo_sb = ac.tile([128, KC * R], F32)
                nc.sync.dma_start(objTo_sb[:].rearrange("p (c n) -> p c n", c=KC),
                                  objT_own.rearrange("(c p) n -> p c n", p=128))
                wsT_sb = ac.tile([128, KC * D], F32)
                nc.sync.dma_start(wsT_sb[:].rearrange("p (c n) -> p c n", c=KC),
                                  wsT.rearrange("(c p) n -> p c n", p=128))
                woT_sb = ac.tile([128, KC * D], F32)
                nc.sync.dma_start(woT_sb[:].rearrange("p (c n) -> p c n", c=KC),
                                  woT.rearrange("(c p) n -> p c n", p=128))
                wsb_sb = ac.tile([1, D], F32)
                nc.sync.dma_start(wsb_sb[:], ws_b[:])
                wob_sb = ac.tile([1, D], F32)
                nc.sync.dma_start(wob_sb[:], wo_b[:])
                ones_sb = ac.tile([1, 128], F32)
                nc.sync.dma_start(ones_sb[:], ones[:])
                for g in range(G):
                    pt = pp.tile([128, D], F32)
                    for c in range(KC):
                        nc.tensor.matmul(
                            pt[:],
                            lhsT=objTo_sb[:, c * R + g * 128: c * R + (g + 1) * 128],
                            rhs=woT_sb[:, c * D:(c + 1) * D],
                            start=(c == 0), stop=False)
                    nc.tensor.matmul(pt[:], lhsT=ones_sb[:1, :], rhs=wob_sb[:1, :],
                                     start=False, stop=True)
                    hi_t = ps.tile([128, D], BF16, tag="hi")
                    nc.vector.tensor_copy(hi_t[:], pt[:])
                    lo_t = ps.tile([128, D], BF16, tag="lo")
                    nc.vector.tensor_tensor(out=lo_t[:], in0=pt[:], in1=hi_t[:],
                                            op=SUB)
                    nc.sync.dma_start(cc_in[g * 128:(g + 1) * 128, :D], hi_t[:])
                    nc.sync.dma_start(cc_in[g * 128:(g + 1) * 128, D:], lo_t[:])
                for g in range(G):
                    pt = pp.tile([128, D], F32)
                    for c in range(KC):
                        nc.tensor.matmul(
                            pt[:],
                            lhsT=objTo_sb[:, c * R + g * 128: c * R + (g + 1) * 128],
                            rhs=wsT_sb[:, c * D:(c + 1) * D],
                            start=(c == 0), stop=False)
                    nc.tensor.matmul(pt[:], lhsT=ones_sb[:1, :], rhs=wsb_sb[:1, :],
                                     start=False, stop=True)
                    nc.vector.tensor_copy(s_hi[:, g * D:(g + 1) * D], pt[:])
                    nc.vector.tensor_tensor(
                        out=s_lo[:, g * D:(g + 1) * D], in0=pt[:],
                        in1=s_hi[:, g * D:(g + 1) * D], op=SUB)

            nc.gpsimd.collective_compute(
                "AllGather", mybir.AluOpType.bypass,
                replica_groups=[list(range(N_CORES))],
                ins=[cc_in[:, :]], outs=[hilo_dram[:, :]])

            # ---------------- stage B ----------------
            with tc.tile_pool(name="small_psum", bufs=2, space="PSUM") as zp, \
                 tc.tile_pool(name="st_psum", bufs=1, space="PSUM") as stp, \
                 tc.tile_pool(name="af_psum", bufs=1, space="PSUM") as afp, \
                 tc.tile_pool(name="tr_psum", bufs=1, space="PSUM") as trp, \
                 tc.tile_pool(name="work", bufs=3) as wk, \
                 tc.tile_pool(name="ohf", bufs=2) as ohf, \
                 tc.tile_pool(name="bigwork", bufs=2) as bw, \
                 tc.tile_pool(name="ohTp", bufs=2) as ohp, \
                 tc.tile_pool(name="fillp", bufs=2) as fp_, \
                 tc.tile_pool(name="expfp", bufs=2) as ep:
                for g in range(G):
                    z_run = wk.tile([128, P], F32, tag="zrun")
                    nc.vector.memset(z_run[:], 0.0)
                    expf_g = ep.tile([128, BB * P], F32, tag="expf")
                    ohTf_g = ohf.tile([128, BB * 128], F32, tag="ohTf")
                    for m in range(MBG):
                        mb = g * MBG + m
                        e0 = mb * MBE
                        uT_t = bw.tile([128, KC * MBE], F32, tag="uT")
                        nc.sync.dma_start(
                            uT_t[:].rearrange("p (c j) -> p c j", c=KC),
                            uT_d[mb, :, :, :].rearrange("c p j -> p c j"))
                        hlT_t = bw.tile([128, 2 * KC, MBE], BF16, tag="hlT")
                        nc.gpsimd.dma_gather(
                            hlT_t[:], hilo_dram[:, :],
                            bpack_sb[:, mb * (MBE // 16):(mb + 1) * (MBE // 16)],
                            MBE, MBE, elem_size=2 * D, transpose=True)
                        oT_t = bw.tile([128, KC * MBE], F32, tag="oT")
                        nc.vector.tensor_tensor(
                            out=oT_t[:].rearrange("p (c j) -> p c j", c=KC),
                            in0=hlT_t[:, 0:KC, :], in1=hlT_t[:, KC:2 * KC, :],
                            op=mybir.AluOpType.add)
                        arb_t = wk.tile([128, MBE], F32, tag="arb")
                        nc.sync.dma_start(
                            arb_t[:],
                            a_row[0:1, e0:e0 + MBE].to_broadcast((128, MBE)))
                        ohT_t = ohp.tile([128, MBE], BF16, tag="ohT")
                        ohT_sl = ohT_t[:]
                        nc.vector.tensor_tensor(
                            out=ohT_sl,
                            in0=rowidx_sb[:, 0:1].to_broadcast([128, MBE]),
                            in1=arb_t[:], op=eq)
                        nc.vector.tensor_tensor(
                            out=ohTf_g[:, m * MBE:(m + 1) * MBE],
                            in0=rowidx_sb[:, 0:1].to_broadcast([128, MBE]),
                            in1=arb_t[:], op=eq)
                        st_ps = stp.tile([128, KC * MBE], F32)
                        for c in range(KC):
                            nc.tensor.matmul(
                                st_ps[:, c * MBE:(c + 1) * MBE],
                                lhsT=s_hi[:, g * D + c * 128: g * D + (c + 1) * 128],
                                rhs=ohT_sl, start=True, stop=False)
                            nc.tensor.matmul(
                                st_ps[:, c * MBE:(c + 1) * MBE],
                                lhsT=s_lo[:, g * D + c * 128: g * D + (c + 1) * 128],
                                rhs=ohT_sl, start=False, stop=True)
                        nc.vector.tensor_mul(oT_t[:], uT_t[:], oT_t[:])
                        prodT_t = bw.tile([128, KC * MBE], F32, tag="prodT")
                        nc.vector.tensor_mul(prodT_t[:], oT_t[:], st_ps[:])
                        af_ps = afp.tile([16, MBE], F32)
                        for c in range(KC):
                            nc.tensor.matmul(
                                af_ps[:],
                                lhsT=wwT_sb[:, c * P:(c + 1) * P],
                                rhs=prodT_t[:, c * MBE:(c + 1) * MBE],
                                start=(c == 0), stop=False)
                        bg_t = wk.tile([2, MBE], F32, tag="bgm")
                        nc.sync.dma_start(bg_t[:], bg_d[:, e0:e0 + MBE])
                        nc.tensor.matmul(af_ps[:], lhsT=bgl_sb[:, :],
                                         rhs=bg_t[:],
                                         start=False, stop=True)
                        expfT_t = wk.tile([16, MBE], F32, tag="expfT")
                        nc.scalar.activation(expfT_t[:], af_ps[:], EXP)
                        etr_ps = trp.tile([128, 4 * P], F32)
                        for q in range(4):
                            nc.tensor.transpose(
                                etr_ps[:, q * P:(q + 1) * P],
                                expfT_t[:, q * 128:(q + 1) * 128], id16_sb[:])
                        nc.scalar.copy(
                            expf_g[:, (m * 4) * P:(m * 4 + 4) * P], etr_ps[:])
                        zmb_ps = zp.tile([128, P], F32, tag="zr")
                        for q in range(4):
                            blk = g * BB + m * 4 + q
                            kk = m * 4 + q
                            oh_t = wk.tile([128, 128], F32, tag="oh")
                            nc.vector.tensor_tensor(
                                out=oh_t[:],
                                in0=a_relT_sb[:, blk:blk + 1].to_broadcast([128, 128]),
                                in1=colidx_sb[:], op=eq)
                            nc.tensor.matmul(
                                zmb_ps[:], lhsT=oh_t[:],
                                rhs=expf_g[:, kk * P:(kk + 1) * P],
                                start=(q == 0), stop=(q == 3))
                        nc.vector.tensor_add(z_run[:], z_run[:], zmb_ps[:])
                    # group tail
                    z_t = wk.tile([128, P], F32, tag="z")
                    nc.vector.tensor_scalar_add(z_t[:], z_run[:], base_sb[:, g:g + 1])
                    recip_t = wk.tile([128, P], F32, tag="recip")
                    nc.vector.reciprocal(recip_t[:], z_t[:])
                    fill_t = fp_.tile([128, 128 * P], F32, tag="fill")
                    nc.vector.tensor_copy(
                        fill_t[:].rearrange("p (j q) -> p j q", q=P),
                        recip_t[:, None, :].broadcast_to([128, 128, P]))
                    out4 = out_ds[g].rearrange("(r a j) q -> r a j q",
                                               a=N // 128, j=128)
                    nc.sync.dma_start(
                        out4[:, :, :, :],
                        fill_t[:].rearrange("p (j q) -> p j q", q=P)[:, None, :, :]
                        .broadcast_to([128, N // 128, 128, P]))
                    for k in range(BB):
                        blk = g * BB + k
                        rg_ps = zp.tile([128, P], F32, tag="zr")
                        nc.tensor.matmul(
                            rg_ps[:], lhsT=ohTf_g[:, k * 128:(k + 1) * 128],
                            rhs=recip_t[:], start=True, stop=True)
                        val_t = wk.tile([128, P], F32, tag="val")
                        nc.vector.tensor_mul(val_t[:], expf_g[:, k * P:(k + 1) * P],
                                             rg_ps[:])
                        nc.gpsimd.indirect_dma_start(
                            out=out_ds[g][:, :],
                            out_offset=bass.IndirectOffsetOnAxis(
                                ap=cellT_sb[:, blk:blk + 1], axis=0),
                            in_=val_t[:], in_offset=None)

    nc.compile()
    return nc


# --------------------------------------------------------------------------
# entry point
# --------------------------------------------------------------------------

def kernel(obj_feats, union_feats, pair_idxs, ws_w, ws_b, wo_w, wo_b, w_w, w_b):
    global LAST_RESULTS
    obj_feats = np.asarray(obj_feats, np.float32)
    union_feats = np.asarray(union_feats, np.float32)
    pair_idxs = np.asarray(pair_idxs)
    ws_w = np.asarray(ws_w, np.float32)
    wo_w = np.asarray(wo_w, np.float32)
    w_w = np.asarray(w_w, np.float32)

    BB, per_core = _host_prep(obj_feats, union_feats, pair_idxs)
    nc = _build_program(BB)

    shared = dict(
        wsT=np.ascontiguousarray(ws_w.T),
        woT=np.ascontiguousarray(wo_w.T),
        wwT=np.ascontiguousarray(w_w.T),
        ws_b=np.asarray(ws_b, np.float32).reshape(1, D),
        wo_b=np.asarray(wo_b, np.float32).reshape(1, D),
        bg_lhs=np.ascontiguousarray(
            np.stack([np.asarray(w_b, np.float32),
                      np.ones(P, np.float32)])),
        ones=np.ones((1, 128), np.float32),
        colidx=np.broadcast_to(np.arange(128, dtype=np.float32), (128, 128)).copy(),
        rowidx=np.arange(128, dtype=np.float32).reshape(128, 1),
        ident16=np.eye(16, dtype=np.float32),
    )
    in_maps = []
    for c in range(N_CORES):
        pc = per_core[c]
        in_maps.append({
            **shared,
            "objT_own": pc["objT_own"],
            "base": pc["base"],
            "uT": pc["uT"],
            "b_pack": pc["b_pack"],
            "a_row": pc["a_row"],
            "a_relT": pc["a_relT"],
            "bg": pc["bg"],
            "cellT": pc["cellT"],
        })

    nc.m = get_hw_module(nc.m)
    res = bass_utils.run_bass_kernel_spmd(nc, in_maps, core_ids=list(range(N_CORES)))
    LAST_RESULTS = res

    out = np.empty((N, N, P), np.float32)
    for c in range(N_CORES):
        for g in range(G):
            out[c * R + g * 128: c * R + (g + 1) * 128] = \
                res.results[c][f"out{g}"].reshape(128, N, P)
    return out
